# revision 25
# baseline (speedup 1.0000x reference)
"""Trainium2 Bass kernel for nn_DualBandDecoder (v2).

Sharding: core pair p = (2p, 2p+1) shares samples (2p, 2p+1); even core
handles bands 0..15, odd core bands 16..30 (+1 dummy slot). Each core
processes 16 band-slots x 2 samples x 2 branches.

Per (slot, hc) the fc1 output for BOTH samples lands in one 2-bank PSUM
tile [128, 1024], so one ACTIVATE tanh covers 1024 elements with a single
per-partition bias (b1 is sample-independent after normalizing features on
DVE). fc1 matmuls are 4-way column-strip tiled (concurrent in the PE
array); fc2 is hc-outer so the 4 bands of a quad overlap. All activation
functions are Tanh/Sin (sigmoid via tanh identity), which live in ONE act
table set -> single table load, assembly pipelines with compute.
"""
import sys
sys.path.insert(0, '/opt/trn_rl_repo')

import numpy as np

import concourse.bacc as bacc
import concourse.tile as tile
import concourse.mybir as mybir
from concourse.bass_utils import run_bass_kernel_spmd

F32 = mybir.dt.float32
FP16 = mybir.dt.float16
I32 = mybir.dt.int32
AF = mybir.ActivationFunctionType
ALU = mybir.AluOpType

# problem constants
B, C, T = 8, 128, 512
BANDS = [2] + [3] * 10 + [8] * 12 + [16] * 7 + [17]
K = len(BANDS)                      # 31
F = sum(BANDS)                      # 257
H = 4 * C                           # 512
NHC = 4
EPS = 1e-5
OFFS = np.concatenate([[0], np.cumsum(BANDS)]).astype(int)

NB = 16                             # band-slots per core
NQ = 4                              # quads per core
T2 = 2 * T                          # 1024: two samples' t-cols

MAGIC = float(1.5 * 2 ** 23)
INV2PI = float(1.0 / (2 * np.pi))
N2PI = float(-2 * np.pi)
PI = float(np.pi)

_cache = {}


def _build():
    nc = bacc.Bacc("TRN2", target_bir_lowering=False)

    ins = {}
    for br in ("m", "p"):
        ins[f"feat_{br}"] = nc.dram_tensor(f"feat_{br}", [C, NB * T2], FP16,
                                           kind="ExternalInput")
        ins[f"w1_{br}"] = nc.dram_tensor(f"w1_{br}", [C, NB * H], FP16,
                                         kind="ExternalInput")
        ins[f"b1pt_{br}"] = nc.dram_tensor(f"b1pt_{br}", [128, NB * NHC], F32,
                                           kind="ExternalInput")
        ins[f"w2s_{br}"] = nc.dram_tensor(f"w2s_{br}", [128, NHC * NB * 32],
                                          FP16, kind="ExternalInput")
        ins[f"b2c_{br}"] = nc.dram_tensor(f"b2c_{br}", [128, NQ], F32,
                                          kind="ExternalInput")
        ins[f"noisy_{br}"] = nc.dram_tensor(f"noisy_{br}", [128, NQ * T2],
                                            FP16, kind="ExternalInput")
    ones_col_d = nc.dram_tensor("ones_col", [128, 1], F32, kind="ExternalInput")
    ones_row_d = nc.dram_tensor("ones_row", [1, 128], F32, kind="ExternalInput")
    halfpi_d = nc.dram_tensor("halfpi", [128, 1], F32, kind="ExternalInput")
    out_d = nc.dram_tensor("out", [128, NQ * 2 * T2], F32,
                           kind="ExternalOutput")

    with tile.TileContext(nc) as tc:
        with (
            tc.tile_pool(name="featq", bufs=3) as feat_pool,
            tc.tile_pool(name="fcq", bufs=2) as fc_pool,
            tc.tile_pool(name="w1q", bufs=2) as w1_pool,
            tc.tile_pool(name="h1sb", bufs=5) as h1_pool,
            tc.tile_pool(name="const", bufs=1) as const_pool,
            tc.tile_pool(name="stats", bufs=2) as st_pool,
            tc.tile_pool(name="asm", bufs=1) as asm_pool,
            tc.tile_pool(name="mainps", bufs=1, space="PSUM") as ps,
        ):
            # ---- single act-table load: silu_and_others has Tanh AND Sin ----
            tl = mybir.InstLoadActFuncSet(
                name=nc.get_next_instruction_name(), act_func_set_id=18,
                ins=[], outs=[])
            nc.scalar.add_instruction(tl)

            # ---- critical path first: quad-0 mag tiles, split DMAs ----
            fq0 = feat_pool.tile([128, 4 * T2], FP16, tag="fq", name="fq_m_0")
            nc.sync.dma_start(fq0[:, 0:2 * T2], ins["feat_m"][:, 0:2 * T2])
            wq0 = w1_pool.tile([128, 4 * H], FP16, tag="w1q", name="w1q_m_0")
            nc.sync.dma_start(wq0[:, 0:2 * H], ins["w1_m"][:, 0:2 * H])
            ones_col = const_pool.tile([128, 1], F32)
            nc.sync.dma_start(ones_col[:], ones_col_d[:])
            ones_row = const_pool.tile([1, 128], F32)
            nc.sync.dma_start(ones_row[:], ones_row_d[:])
            b1pt_m = const_pool.tile([128, NB * NHC], F32, tag="b1_m")
            nc.sync.dma_start(b1pt_m[:], ins["b1pt_m"][:])
            nc.sync.dma_start(fq0[:, 2 * T2:4 * T2],
                              ins["feat_m"][:, 2 * T2:4 * T2])
            nc.sync.dma_start(wq0[:, 2 * H:4 * H], ins["w1_m"][:, 2 * H:4 * H])

            # ---- remaining constants ----
            halfpi = const_pool.tile([128, 1], F32)
            nc.sync.dma_start(halfpi[:], halfpi_d[:])
            cb = {}
            for br in ("m", "p"):
                if br == "m":
                    b1pt = b1pt_m
                else:
                    b1pt = const_pool.tile([128, NB * NHC], F32, tag="b1_p")
                    nc.sync.dma_start(b1pt[:], ins["b1pt_p"][:])
                w2s = const_pool.tile([128, NHC * NB * 32], FP16, tag=f"w2_{br}")
                nc.sync.dma_start(w2s[:], ins[f"w2s_{br}"][:])
                b2c = const_pool.tile([128, NQ], F32, tag=f"b2_{br}")
                nc.sync.dma_start(b2c[:], ins[f"b2c_{br}"][:])
                noisy = const_pool.tile([128, NQ * T2], FP16, tag=f"no_{br}")
                nc.gpsimd.dma_start(noisy[:], ins[f"noisy_{br}"][:])
                cb[br] = (b1pt, w2s, b2c, noisy)

            # ---- PE warm-up ----
            for wi in range(16):
                wps = ps.tile([128, T2], F32, tag="ps2", bufs=3,
                              name=f"warm_{wi}")
                nc.tensor.matmul(wps[:, 0:T], wq0[:, 0:128], wq0[:, 0:T],
                                 start=True, stop=True)

            grp = {}

            def load_block(q, br):
                if q == 0 and br == "m":
                    return fq0, wq0
                fq = feat_pool.tile([128, 4 * T2], FP16, tag="fq",
                                    name=f"fq_{br}_{q}")
                nc.sync.dma_start(
                    fq[:], ins[f"feat_{br}"][:, q * 4 * T2:(q + 1) * 4 * T2])
                wq = w1_pool.tile([128, 4 * H], FP16, tag="w1q",
                                  name=f"w1q_{br}_{q}")
                nc.sync.dma_start(
                    wq[:], ins[f"w1_{br}"][:, q * 4 * H:(q + 1) * 4 * H])
                return fq, wq

            def stats_fcent(q, br, fq):
                st = st_pool.tile([128, 8 * 6], F32, tag="st",
                                  name=f"st_{br}_{q}")
                ag = st_pool.tile([128, 8 * 2], F32, tag="ag",
                                  name=f"ag_{br}_{q}")
                for u in range(8):
                    r, s = u // 2, u % 2
                    nc.vector.bn_stats(st[:, u * 6:(u + 1) * 6],
                                       fq[:, r * T2 + s * T: r * T2 + (s + 1) * T])
                    nc.vector.bn_aggr(ag[:, u * 2:(u + 1) * 2],
                                      st[:, u * 6:(u + 1) * 6])
                ag3 = ag[:].rearrange("c (u two) -> c u two", two=2)
                mean_ap = ag3[:, :, 0]
                var_ap = ag3[:, :, 1]
                sums = st_pool.tile([128, 16], F32, tag="sums",
                                    name=f"sums_{br}_{q}")
                nc.vector.tensor_copy(sums[:, 0:8], mean_ap)
                tmp = st_pool.tile([128, 8], F32, tag="tmp",
                                   name=f"tmp_{br}_{q}")
                nc.vector.tensor_mul(tmp[:], mean_ap, mean_ap)
                nc.vector.tensor_add(sums[:, 8:16], tmp[:], var_ap)
                ps_s = ps.tile([1, 16], F32, tag="smalls", bufs=2,
                               name=f"pss_{br}_{q}")
                nc.tensor.matmul(ps_s[:], ones_col[:], sums[:],
                                 start=True, stop=True)
                g = st_pool.tile([1, 16], F32, tag="g", name=f"g_{br}_{q}")
                nc.vector.tensor_scalar_mul(g[:], ps_s[:], 1.0 / C)
                gm2 = st_pool.tile([1, 8], F32, tag="gm2", name=f"gm2_{br}_{q}")
                nc.vector.tensor_mul(gm2[:], g[:, 0:8], g[:, 0:8])
                vv = st_pool.tile([1, 8], F32, tag="vv", name=f"vv_{br}_{q}")
                nc.vector.tensor_sub(vv[:], g[:, 8:16], gm2[:])
                nc.vector.tensor_scalar_add(vv[:], vv[:], EPS)
                yy = st_pool.tile([1, 8], F32, tag="yy", name=f"yy_{br}_{q}")
                nc.vector.tensor_scalar(yy[:].bitcast(I32), vv[:].bitcast(I32),
                                        1, -1, op0=ALU.arith_shift_right,
                                        op1=ALU.bitwise_xor)
                nc.vector.tensor_scalar_add(yy[:].bitcast(I32),
                                            yy[:].bitcast(I32), 0x5f3759e0)
                invmean = st_pool.tile([1, 16], F32, tag="invmean",
                                       name=f"im_{br}_{q}")
                tnr = st_pool.tile([1, 8], F32, tag="tnr", name=f"tnr_{br}_{q}")
                for it in range(2):
                    nc.vector.tensor_mul(tnr[:], yy[:], yy[:])
                    nc.vector.tensor_mul(tnr[:], tnr[:], vv[:])
                    nc.vector.tensor_scalar(tnr[:], tnr[:], -0.5, 1.5,
                                            op0=ALU.mult, op1=ALU.add)
                    dst = yy[:] if it < 1 else invmean[:, 0:8]
                    nc.vector.tensor_mul(dst, yy[:], tnr[:])
                nc.vector.tensor_copy(invmean[:, 8:16], g[:, 0:8])
                ps_b = ps.tile([128, 16], F32, tag="smalls", bufs=2,
                               name=f"psb_{br}_{q}")
                nc.tensor.matmul(ps_b[:], ones_row[:], invmean[:],
                                 start=True, stop=True)
                bbq = st_pool.tile([128, 16], F32, tag="bbq", bufs=3,
                                   name=f"bbq_{br}_{q}")
                nc.vector.tensor_copy(bbq[:], ps_b[:])
                fcq = fc_pool.tile([128, 4 * T2], FP16, tag="fcq",
                                   name=f"fcq_{br}_{q}")
                for u in range(8):
                    r, s = u // 2, u % 2
                    nc.vector.tensor_scalar(
                        fcq[:, r * T2 + s * T: r * T2 + (s + 1) * T],
                        fq[:, r * T2 + s * T: r * T2 + (s + 1) * T],
                        bbq[:, 8 + u:9 + u], bbq[:, u:u + 1],
                        op0=ALU.subtract, op1=ALU.mult)
                return fcq

            def stats_fcent0(fq):
                """Block (0,m): stats in two 2-slot groups so fc1 of slot 0
                starts before slots 2-3 features even land."""
                fcq = fc_pool.tile([128, 4 * T2], FP16, tag="fcq",
                                   name="fcq_m_0")
                for gi in range(2):
                    st = st_pool.tile([128, 4 * 6], F32, tag="st",
                                      name=f"st0_{gi}")
                    ag = st_pool.tile([128, 4 * 2], F32, tag="ag",
                                      name=f"ag0_{gi}")
                    for v in range(4):
                        u = gi * 4 + v
                        r, s = u // 2, u % 2
                        nc.vector.bn_stats(st[:, v * 6:(v + 1) * 6],
                                           fq[:, r * T2 + s * T: r * T2 + (s + 1) * T])
                        nc.vector.bn_aggr(ag[:, v * 2:(v + 1) * 2],
                                          st[:, v * 6:(v + 1) * 6])
                    ag3 = ag[:].rearrange("c (u two) -> c u two", two=2)
                    mean_ap = ag3[:, :, 0]
                    var_ap = ag3[:, :, 1]
                    sums = st_pool.tile([128, 8], F32, tag="sums",
                                        name=f"sums0_{gi}")
                    nc.vector.tensor_copy(sums[:, 0:4], mean_ap)
                    tmp = st_pool.tile([128, 4], F32, tag="tmp",
                                       name=f"tmp0_{gi}")
                    nc.vector.tensor_mul(tmp[:], mean_ap, mean_ap)
                    nc.vector.tensor_add(sums[:, 4:8], tmp[:], var_ap)
                    ps_s = ps.tile([1, 8], F32, tag="smalls", bufs=2,
                                   name=f"pss0_{gi}")
                    nc.tensor.matmul(ps_s[:], ones_col[:], sums[:],
                                     start=True, stop=True)
                    g = st_pool.tile([1, 8], F32, tag="g", name=f"g0_{gi}")
                    nc.vector.tensor_scalar_mul(g[:], ps_s[:], 1.0 / C)
                    gm2 = st_pool.tile([1, 4], F32, tag="gm2",
                                       name=f"gm20_{gi}")
                    nc.vector.tensor_mul(gm2[:], g[:, 0:4], g[:, 0:4])
                    vv = st_pool.tile([1, 4], F32, tag="vv", name=f"vv0_{gi}")
                    nc.vector.tensor_sub(vv[:], g[:, 4:8], gm2[:])
                    nc.vector.tensor_scalar_add(vv[:], vv[:], EPS)
                    yy = st_pool.tile([1, 4], F32, tag="yy", name=f"yy0_{gi}")
                    nc.vector.tensor_scalar(yy[:].bitcast(I32),
                                            vv[:].bitcast(I32),
                                            1, -1, op0=ALU.arith_shift_right,
                                            op1=ALU.bitwise_xor)
                    nc.vector.tensor_scalar_add(yy[:].bitcast(I32),
                                                yy[:].bitcast(I32), 0x5f3759e0)
                    invmean = st_pool.tile([1, 8], F32, tag="invmean",
                                           name=f"im0_{gi}")
                    tnr = st_pool.tile([1, 4], F32, tag="tnr",
                                       name=f"tnr0_{gi}")
                    for it in range(2):
                        nc.vector.tensor_mul(tnr[:], yy[:], yy[:])
                        nc.vector.tensor_mul(tnr[:], tnr[:], vv[:])
                        nc.vector.tensor_scalar(tnr[:], tnr[:], -0.5, 1.5,
                                                op0=ALU.mult, op1=ALU.add)
                        dst = yy[:] if it < 1 else invmean[:, 0:4]
                        nc.vector.tensor_mul(dst, yy[:], tnr[:])
                    nc.vector.tensor_copy(invmean[:, 4:8], g[:, 0:4])
                    ps_b = ps.tile([128, 8], F32, tag="smalls", bufs=2,
                                   name=f"psb0_{gi}")
                    nc.tensor.matmul(ps_b[:], ones_row[:], invmean[:],
                                     start=True, stop=True)
                    bbq = st_pool.tile([128, 8], F32, tag="bbq", bufs=3,
                                       name=f"bbq0_{gi}")
                    nc.vector.tensor_copy(bbq[:], ps_b[:])
                    for v in range(4):
                        u = gi * 4 + v
                        r, s = u // 2, u % 2
                        nc.vector.tensor_scalar(
                            fcq[:, r * T2 + s * T: r * T2 + (s + 1) * T],
                            fq[:, r * T2 + s * T: r * T2 + (s + 1) * T],
                            bbq[:, 4 + v:5 + v], bbq[:, v:v + 1],
                            op0=ALU.subtract, op1=ALU.mult)
                return fcq

            def fc_block(q, br, fcq, wq):
                b1pt, w2s, b2c, _ = cb[br]
                h1s = []
                for r in range(4):
                    slot = q * 4 + r
                    h1 = h1_pool.tile([128, NHC * T2], FP16, tag="h1")
                    h1s.append(h1)
                    for hc in range(NHC):
                        p2 = ps.tile([128, T2], F32, tag="ps2", bufs=3,
                                     name=f"p2_{br}_{slot}_{hc}")
                        for s in range(2):
                            for j in range(4):
                                nc.tensor.matmul(
                                    p2[32 * j:32 * j + 32, s * T:(s + 1) * T],
                                    wq[:, (r * NHC + hc) * 128 + 32 * j:
                                          (r * NHC + hc) * 128 + 32 * j + 32],
                                    fcq[:, r * T2 + s * T: r * T2 + (s + 1) * T],
                                    start=True, stop=True,
                                    tile_position=(0, 32 * j))
                        nc.scalar.activation(
                            h1[:, hc * T2:(hc + 1) * T2], p2[:], AF.Tanh,
                            bias=b1pt[:, slot * NHC + hc: slot * NHC + hc + 1])
                fq2 = ps.tile([128, T2], F32, tag="ps2", bufs=3,
                              name=f"fq2_{br}_{q}")
                for hc in range(NHC):
                    for r in range(4):
                        slot = q * 4 + r
                        for s in range(2):
                            nc.tensor.matmul(
                                fq2[32 * r:32 * r + 32, s * T:(s + 1) * T],
                                w2s[:, hc * NB * 32 + slot * 32:
                                       hc * NB * 32 + slot * 32 + 32],
                                h1s[r][:, hc * T2 + s * T: hc * T2 + (s + 1) * T],
                                start=(hc == 0), stop=(hc == NHC - 1),
                                tile_position=(0, 32 * r))
                gt = const_pool.tile([128, T2], FP16, tag=f"grp_{br}_{q}",
                                     name=f"grp_{br}_{q}")
                nc.scalar.activation(gt[:], fq2[:], AF.Tanh,
                                     bias=b2c[:, q:q + 1],
                                     scale=0.5 if br == "m" else 1.0)
                grp[(q, br)] = gt

            def assembly(q):
                _, _, _, noisy_m = cb["m"]
                _, _, _, noisy_p = cb["p"]
                nmag = noisy_m[:, q * T2:(q + 1) * T2]
                nph = noisy_p[:, q * T2:(q + 1) * T2]
                mask01 = asm_pool.tile([128, T2], FP16, tag="mask01")
                nc.vector.tensor_scalar(mask01[:], grp[(q, "m")][:], 0.5, 0.5,
                                        op0=ALU.mult, op1=ALU.add)
                enh = asm_pool.tile([128, T2], FP16, tag="enh")
                nc.vector.tensor_mul(enh[:], mask01[:], nmag)
                ang = asm_pool.tile([128, T2], F32, tag="ang")
                nc.vector.scalar_tensor_tensor(ang[:], grp[(q, "p")][:], PI,
                                               nph, op0=ALU.mult, op1=ALU.add)
                t2 = asm_pool.tile([128, T2], F32, tag="t2")
                nc.vector.tensor_scalar(t2[:], ang[:], INV2PI, MAGIC,
                                        op0=ALU.mult, op1=ALU.add)
                m2 = asm_pool.tile([128, T2], F32, tag="m2")
                nc.vector.tensor_scalar(m2[:], t2[:], MAGIC, N2PI,
                                        op0=ALU.subtract, op1=ALU.mult)
                ws = asm_pool.tile([128, T2], F32, tag="ws")
                nc.vector.tensor_add(ws[:], m2[:], ang[:])
                sn = asm_pool.tile([128, T2], FP16, tag="sn")
                nc.scalar.activation(sn[:], ws[:], AF.Sin)
                # cos(ws) = sin(pi/2 - |ws|); |ws| <= pi keeps Sin in-domain
                aws = asm_pool.tile([128, T2], F32, tag="aws")
                nc.vector.tensor_scalar(aws[:].bitcast(I32), ws[:].bitcast(I32),
                                        0x7FFFFFFF, 0,
                                        op0=ALU.bitwise_and, op1=ALU.bitwise_or)
                cn = asm_pool.tile([128, T2], FP16, tag="cn")
                nc.scalar.activation(cn[:], aws[:], AF.Sin, bias=halfpi[:],
                                     scale=-1.0)
                ot = asm_pool.tile([128, 2 * T2], F32, tag="ot")
                ot2 = ot[:].rearrange("p (st two) -> p st two", two=2)
                nc.vector.tensor_mul(ot2[:, :, 0], enh[:], cn[:])
                nc.vector.tensor_mul(ot2[:, :, 1], enh[:], sn[:])
                nc.sync.dma_start(out_d[:, q * 2 * T2:(q + 1) * 2 * T2], ot[:])

            # ---- software-pipelined main loop ----
            blocks = [(q, br) for q in range(NQ) for br in ("m", "p")]
            tiles = {0: load_block(*blocks[0])}
            fcqs = {}
            fcqs[0] = stats_fcent0(tiles[0][0])
            for i, (q, br) in enumerate(blocks):
                if i + 1 < len(blocks):
                    tiles[i + 1] = load_block(*blocks[i + 1])
                fc_block(q, br, fcqs[i], tiles[i][1])
                if i + 1 < len(blocks):
                    fcqs[i + 1] = stats_fcent(blocks[i + 1][0],
                                              blocks[i + 1][1],
                                              tiles[i + 1][0])
                # assembly of quad q-1 deferred: its DVE chain and Sin ops
                # hide under this block's tanh stream
                if br == "m" and q >= 1:
                    assembly(q - 1)
            assembly(NQ - 1)

    nc.compile()
    return nc


def _prep_core(bands, gamma, beta, W1, b1, W2, b2, halve_b2):
    """Host-side constant prep for one (core, branch). bands: list of global
    band indices (len<=16; missing slots are zero/dummy)."""
    nb = len(bands)
    W1g = W1 * gamma[:, None, :]                     # [K, H, C]
    w1 = np.zeros((C, NB * H), np.float16)
    for sl, k in enumerate(bands):
        w1[:, sl * H:(sl + 1) * H] = W1g[k].T.astype(np.float16)
    b1p = b1 + np.einsum('khc,kc->kh', W1, beta)     # [K, H]
    b1pt = np.zeros((128, NB * NHC), np.float32)
    for sl, k in enumerate(bands):
        for hc in range(NHC):
            b1pt[:, sl * NHC + hc] = b1p[k, hc * 128:(hc + 1) * 128]
    w2s = np.zeros((128, NHC * NB * 32), np.float16)
    for sl, k in enumerate(bands):
        w, off = BANDS[k], int(OFFS[k])
        for hc in range(NHC):
            w2s[:, hc * NB * 32 + sl * 32: hc * NB * 32 + sl * 32 + w] = \
                W2[off:off + w, hc * 128:(hc + 1) * 128].T.astype(np.float16)
    b2c = np.zeros((128, NQ), np.float32)
    for sl, k in enumerate(bands):
        qq, r = sl // 4, sl % 4
        w, off = BANDS[k], int(OFFS[k])
        b2c[32 * r:32 * r + w, qq] = b2[off:off + w]
    if halve_b2:
        b2c *= 0.5
    return w1, b1pt, w2s, b2c


def _noisy_strips(bands, arr2):
    """arr2: [2, F, T] noisy for the pair's two samples -> strip layout
    [128, NQ*T2] fp16."""
    outp = np.zeros((128, NQ * T2), np.float16)
    for sl, k in enumerate(bands):
        qq, r = sl // 4, sl % 4
        w, off = BANDS[k], int(OFFS[k])
        for s in range(2):
            outp[32 * r:32 * r + w, qq * T2 + s * T:qq * T2 + (s + 1) * T] = \
                arr2[s, off:off + w, :].astype(np.float16)
    return outp


def kernel(mag_features, phase_features, noisy_mag, noisy_phase,
           mag_gamma, mag_beta, mag_W1, mag_b1, mag_W2, mag_b2,
           ph_gamma, ph_beta, ph_W1, ph_b1, ph_W2, ph_b2):
    if "nc" not in _cache:
        _cache["nc"] = _build()
    nc = _cache["nc"]

    mag_features = np.asarray(mag_features)
    phase_features = np.asarray(phase_features)
    noisy_mag = np.asarray(noisy_mag)
    noisy_phase = np.asarray(noisy_phase)
    prm = {
        "m": tuple(np.asarray(x) for x in
                   (mag_gamma, mag_beta, mag_W1, mag_b1, mag_W2, mag_b2)),
        "p": tuple(np.asarray(x) for x in
                   (ph_gamma, ph_beta, ph_W1, ph_b1, ph_W2, ph_b2)),
    }
    feats = {"m": mag_features, "p": phase_features}
    noisy = {"m": noisy_mag, "p": noisy_phase}

    band_sets = [list(range(0, 16)), list(range(16, 31))]
    shared = dict(
        ones_col=np.ones((128, 1), np.float32),
        ones_row=np.ones((1, 128), np.float32),
        halfpi=np.full((128, 1), np.pi / 2, np.float32),
    )
    # per-(half, branch) weight prep (same for every pair)
    wprep = {}
    for half in range(2):
        for br in ("m", "p"):
            g, be, W1_, b1_, W2_, b2_ = prm[br]
            wprep[(half, br)] = _prep_core(band_sets[half], g, be, W1_, b1_,
                                           W2_, b2_, halve_b2=(br == "m"))

    in_maps = []
    for core in range(8):
        pair, half = core // 2, core % 2
        sA, sB = 2 * pair, 2 * pair + 1
        bands = band_sets[half]
        m = dict(shared)
        for br in ("m", "p"):
            w1, b1pt, w2s, b2c = wprep[(half, br)]
            m[f"w1_{br}"], m[f"b1pt_{br}"] = w1, b1pt
            m[f"w2s_{br}"], m[f"b2c_{br}"] = w2s, b2c
            # features: [2, C, T, nb] -> [C, nb, 2, T] -> [128, NB*T2]
            fa = feats[br][[sA, sB]][:, :, :, bands]        # [2,C,T,nb]
            fa = np.ascontiguousarray(fa.transpose(1, 3, 0, 2))  # [C,nb,2,T]
            ft = np.zeros((C, NB * T2), np.float16)
            ft[:, :fa.shape[1] * T2] = fa.reshape(C, -1).astype(np.float16)
            m[f"feat_{br}"] = ft
            m[f"noisy_{br}"] = _noisy_strips(bands, noisy[br][[sA, sB]])
        in_maps.append(m)

    import os
    trace = bool(os.environ.get("BASS_PROFILE"))
    res = run_bass_kernel_spmd(nc, in_maps, list(range(8)), trace=trace)
    _cache["last_result"] = res

    out = np.zeros((B, F, T), np.complex64)
    for core in range(8):
        pair, half = core // 2, core % 2
        sA, sB = 2 * pair, 2 * pair + 1
        bands = band_sets[half]
        oc = res.results[core]["out"].reshape(128, NQ, 2, T, 2)
        occ = (oc[..., 0] + 1j * oc[..., 1]).astype(np.complex64)
        for sl, k in enumerate(bands):
            qq, r = sl // 4, sl % 4
            w, off = BANDS[k], int(OFFS[k])
            out[sA, off:off + w] = occ[32 * r:32 * r + w, qq, 0]
            out[sB, off:off + w] = occ[32 * r:32 * r + w, qq, 1]
    return out


# revision 26
# speedup vs baseline: 1.0028x; 1.0028x over previous
"""Trainium2 Bass kernel for nn_DualBandDecoder (v2).

Sharding: core pair p = (2p, 2p+1) shares samples (2p, 2p+1); even core
handles bands 0..15, odd core bands 16..30 (+1 dummy slot). Each core
processes 16 band-slots x 2 samples x 2 branches.

Per (slot, hc) the fc1 output for BOTH samples lands in one 2-bank PSUM
tile [128, 1024], so one ACTIVATE tanh covers 1024 elements with a single
per-partition bias (b1 is sample-independent after normalizing features on
DVE). fc1 matmuls are 4-way column-strip tiled (concurrent in the PE
array); fc2 is hc-outer so the 4 bands of a quad overlap. All activation
functions are Tanh/Sin (sigmoid via tanh identity), which live in ONE act
table set -> single table load, assembly pipelines with compute.
"""
import sys
sys.path.insert(0, '/opt/trn_rl_repo')

import numpy as np

import concourse.bacc as bacc
import concourse.tile as tile
import concourse.mybir as mybir
from concourse.bass_utils import run_bass_kernel_spmd

F32 = mybir.dt.float32
FP16 = mybir.dt.float16
I32 = mybir.dt.int32
AF = mybir.ActivationFunctionType
ALU = mybir.AluOpType

# problem constants
B, C, T = 8, 128, 512
BANDS = [2] + [3] * 10 + [8] * 12 + [16] * 7 + [17]
K = len(BANDS)                      # 31
F = sum(BANDS)                      # 257
H = 4 * C                           # 512
NHC = 4
EPS = 1e-5
OFFS = np.concatenate([[0], np.cumsum(BANDS)]).astype(int)

NB = 16                             # band-slots per core
NQ = 4                              # quads per core
T2 = 2 * T                          # 1024: two samples' t-cols

MAGIC = float(1.5 * 2 ** 23)
INV2PI = float(1.0 / (2 * np.pi))
N2PI = float(-2 * np.pi)
PI = float(np.pi)

_cache = {}


def _build():
    nc = bacc.Bacc("TRN2", target_bir_lowering=False)

    ins = {}
    for br in ("m", "p"):
        ins[f"feat_{br}"] = nc.dram_tensor(f"feat_{br}", [C, NB * T2], FP16,
                                           kind="ExternalInput")
        ins[f"w1_{br}"] = nc.dram_tensor(f"w1_{br}", [C, NB * H], FP16,
                                         kind="ExternalInput")
        ins[f"b1pt_{br}"] = nc.dram_tensor(f"b1pt_{br}", [128, NB * NHC], F32,
                                           kind="ExternalInput")
        ins[f"w2s_{br}"] = nc.dram_tensor(f"w2s_{br}", [128, NHC * NB * 32],
                                          FP16, kind="ExternalInput")
        ins[f"b2c_{br}"] = nc.dram_tensor(f"b2c_{br}", [128, NQ], F32,
                                          kind="ExternalInput")
        ins[f"noisy_{br}"] = nc.dram_tensor(f"noisy_{br}", [128, NQ * T2],
                                            FP16, kind="ExternalInput")
    ones_col_d = nc.dram_tensor("ones_col", [128, 1], F32, kind="ExternalInput")
    ones_row_d = nc.dram_tensor("ones_row", [1, 128], F32, kind="ExternalInput")
    halfpi_d = nc.dram_tensor("halfpi", [128, 1], F32, kind="ExternalInput")
    out_d = nc.dram_tensor("out", [128, NQ * 2 * T2], F32,
                           kind="ExternalOutput")

    with tile.TileContext(nc) as tc:
        with (
            tc.tile_pool(name="featq", bufs=3) as feat_pool,
            tc.tile_pool(name="fcq", bufs=2) as fc_pool,
            tc.tile_pool(name="w1q", bufs=2) as w1_pool,
            tc.tile_pool(name="h1sb", bufs=5) as h1_pool,
            tc.tile_pool(name="const", bufs=1) as const_pool,
            tc.tile_pool(name="stats", bufs=2) as st_pool,
            tc.tile_pool(name="asm", bufs=1) as asm_pool,
            tc.tile_pool(name="mainps", bufs=1, space="PSUM") as ps,
        ):
            # ---- single act-table load: silu_and_others has Tanh AND Sin ----
            tl = mybir.InstLoadActFuncSet(
                name=nc.get_next_instruction_name(), act_func_set_id=18,
                ins=[], outs=[])
            nc.scalar.add_instruction(tl)

            # ---- critical path first: quad-0 mag tiles, split DMAs on
            # two queues (sync: features, gpsimd: weights) ----
            fq0 = feat_pool.tile([128, 4 * T2], FP16, tag="fq", name="fq_m_0")
            nc.sync.dma_start(fq0[:, 0:2 * T2], ins["feat_m"][:, 0:2 * T2])
            wq0 = w1_pool.tile([128, 4 * H], FP16, tag="w1q", name="w1q_m_0")
            nc.gpsimd.dma_start(wq0[:, 0:2 * H], ins["w1_m"][:, 0:2 * H])
            ones_col = const_pool.tile([128, 1], F32)
            nc.sync.dma_start(ones_col[:], ones_col_d[:])
            ones_row = const_pool.tile([1, 128], F32)
            nc.sync.dma_start(ones_row[:], ones_row_d[:])
            b1pt_m = const_pool.tile([128, NB * NHC], F32, tag="b1_m")
            nc.gpsimd.dma_start(b1pt_m[:], ins["b1pt_m"][:])
            nc.sync.dma_start(fq0[:, 2 * T2:4 * T2],
                              ins["feat_m"][:, 2 * T2:4 * T2])
            nc.gpsimd.dma_start(wq0[:, 2 * H:4 * H], ins["w1_m"][:, 2 * H:4 * H])

            # ---- remaining constants ----
            halfpi = const_pool.tile([128, 1], F32)
            nc.sync.dma_start(halfpi[:], halfpi_d[:])
            cb = {}
            for br in ("m", "p"):
                if br == "m":
                    b1pt = b1pt_m
                else:
                    b1pt = const_pool.tile([128, NB * NHC], F32, tag="b1_p")
                    nc.sync.dma_start(b1pt[:], ins["b1pt_p"][:])
                w2s = const_pool.tile([128, NHC * NB * 32], FP16, tag=f"w2_{br}")
                nc.sync.dma_start(w2s[:], ins[f"w2s_{br}"][:])
                b2c = const_pool.tile([128, NQ], F32, tag=f"b2_{br}")
                nc.sync.dma_start(b2c[:], ins[f"b2c_{br}"][:])
                noisy = const_pool.tile([128, NQ * T2], FP16, tag=f"no_{br}")
                nc.gpsimd.dma_start(noisy[:], ins[f"noisy_{br}"][:])
                cb[br] = (b1pt, w2s, b2c, noisy)

            grp = {}

            def load_block(q, br):
                if q == 0 and br == "m":
                    return fq0, wq0
                fq = feat_pool.tile([128, 4 * T2], FP16, tag="fq",
                                    name=f"fq_{br}_{q}")
                nc.sync.dma_start(
                    fq[:], ins[f"feat_{br}"][:, q * 4 * T2:(q + 1) * 4 * T2])
                wq = w1_pool.tile([128, 4 * H], FP16, tag="w1q",
                                  name=f"w1q_{br}_{q}")
                nc.sync.dma_start(
                    wq[:], ins[f"w1_{br}"][:, q * 4 * H:(q + 1) * 4 * H])
                return fq, wq

            def stats_fcent(q, br, fq):
                st = st_pool.tile([128, 8 * 6], F32, tag="st",
                                  name=f"st_{br}_{q}")
                ag = st_pool.tile([128, 8 * 2], F32, tag="ag",
                                  name=f"ag_{br}_{q}")
                for u in range(8):
                    r, s = u // 2, u % 2
                    nc.vector.bn_stats(st[:, u * 6:(u + 1) * 6],
                                       fq[:, r * T2 + s * T: r * T2 + (s + 1) * T])
                    nc.vector.bn_aggr(ag[:, u * 2:(u + 1) * 2],
                                      st[:, u * 6:(u + 1) * 6])
                ag3 = ag[:].rearrange("c (u two) -> c u two", two=2)
                mean_ap = ag3[:, :, 0]
                var_ap = ag3[:, :, 1]
                sums = st_pool.tile([128, 16], F32, tag="sums",
                                    name=f"sums_{br}_{q}")
                nc.vector.tensor_copy(sums[:, 0:8], mean_ap)
                tmp = st_pool.tile([128, 8], F32, tag="tmp",
                                   name=f"tmp_{br}_{q}")
                nc.vector.tensor_mul(tmp[:], mean_ap, mean_ap)
                nc.vector.tensor_add(sums[:, 8:16], tmp[:], var_ap)
                ps_s = ps.tile([1, 16], F32, tag="smalls", bufs=2,
                               name=f"pss_{br}_{q}")
                nc.tensor.matmul(ps_s[:], ones_col[:], sums[:],
                                 start=True, stop=True)
                g = st_pool.tile([1, 16], F32, tag="g", name=f"g_{br}_{q}")
                nc.vector.tensor_scalar_mul(g[:], ps_s[:], 1.0 / C)
                gm2 = st_pool.tile([1, 8], F32, tag="gm2", name=f"gm2_{br}_{q}")
                nc.vector.tensor_mul(gm2[:], g[:, 0:8], g[:, 0:8])
                vv = st_pool.tile([1, 8], F32, tag="vv", name=f"vv_{br}_{q}")
                nc.vector.tensor_sub(vv[:], g[:, 8:16], gm2[:])
                nc.vector.tensor_scalar_add(vv[:], vv[:], EPS)
                yy = st_pool.tile([1, 8], F32, tag="yy", name=f"yy_{br}_{q}")
                nc.vector.tensor_scalar(yy[:].bitcast(I32), vv[:].bitcast(I32),
                                        1, -1, op0=ALU.arith_shift_right,
                                        op1=ALU.bitwise_xor)
                nc.vector.tensor_scalar_add(yy[:].bitcast(I32),
                                            yy[:].bitcast(I32), 0x5f3759e0)
                invmean = st_pool.tile([1, 16], F32, tag="invmean",
                                       name=f"im_{br}_{q}")
                tnr = st_pool.tile([1, 8], F32, tag="tnr", name=f"tnr_{br}_{q}")
                for it in range(2):
                    nc.vector.tensor_mul(tnr[:], yy[:], yy[:])
                    nc.vector.tensor_mul(tnr[:], tnr[:], vv[:])
                    nc.vector.tensor_scalar(tnr[:], tnr[:], -0.5, 1.5,
                                            op0=ALU.mult, op1=ALU.add)
                    dst = yy[:] if it < 1 else invmean[:, 0:8]
                    nc.vector.tensor_mul(dst, yy[:], tnr[:])
                nc.vector.tensor_copy(invmean[:, 8:16], g[:, 0:8])
                ps_b = ps.tile([128, 16], F32, tag="smalls", bufs=2,
                               name=f"psb_{br}_{q}")
                nc.tensor.matmul(ps_b[:], ones_row[:], invmean[:],
                                 start=True, stop=True)
                bbq = st_pool.tile([128, 16], F32, tag="bbq", bufs=3,
                                   name=f"bbq_{br}_{q}")
                nc.vector.tensor_copy(bbq[:], ps_b[:])
                fcq = fc_pool.tile([128, 4 * T2], FP16, tag="fcq",
                                   name=f"fcq_{br}_{q}")
                for u in range(8):
                    r, s = u // 2, u % 2
                    nc.vector.tensor_scalar(
                        fcq[:, r * T2 + s * T: r * T2 + (s + 1) * T],
                        fq[:, r * T2 + s * T: r * T2 + (s + 1) * T],
                        bbq[:, 8 + u:9 + u], bbq[:, u:u + 1],
                        op0=ALU.subtract, op1=ALU.mult)
                return fcq

            def stats_fcent0(fq):
                """Block (0,m): stats in two 2-slot groups so fc1 of slot 0
                starts before slots 2-3 features even land."""
                fcq = fc_pool.tile([128, 4 * T2], FP16, tag="fcq",
                                   name="fcq_m_0")
                for gi in range(2):
                    st = st_pool.tile([128, 4 * 6], F32, tag="st",
                                      name=f"st0_{gi}")
                    ag = st_pool.tile([128, 4 * 2], F32, tag="ag",
                                      name=f"ag0_{gi}")
                    for v in range(4):
                        u = gi * 4 + v
                        r, s = u // 2, u % 2
                        nc.vector.bn_stats(st[:, v * 6:(v + 1) * 6],
                                           fq[:, r * T2 + s * T: r * T2 + (s + 1) * T])
                        nc.vector.bn_aggr(ag[:, v * 2:(v + 1) * 2],
                                          st[:, v * 6:(v + 1) * 6])
                    ag3 = ag[:].rearrange("c (u two) -> c u two", two=2)
                    mean_ap = ag3[:, :, 0]
                    var_ap = ag3[:, :, 1]
                    sums = st_pool.tile([128, 8], F32, tag="sums",
                                        name=f"sums0_{gi}")
                    nc.vector.tensor_copy(sums[:, 0:4], mean_ap)
                    tmp = st_pool.tile([128, 4], F32, tag="tmp",
                                       name=f"tmp0_{gi}")
                    nc.vector.tensor_mul(tmp[:], mean_ap, mean_ap)
                    nc.vector.tensor_add(sums[:, 4:8], tmp[:], var_ap)
                    ps_s = ps.tile([1, 8], F32, tag="smalls", bufs=2,
                                   name=f"pss0_{gi}")
                    nc.tensor.matmul(ps_s[:], ones_col[:], sums[:],
                                     start=True, stop=True)
                    g = st_pool.tile([1, 8], F32, tag="g", name=f"g0_{gi}")
                    nc.vector.tensor_scalar_mul(g[:], ps_s[:], 1.0 / C)
                    gm2 = st_pool.tile([1, 4], F32, tag="gm2",
                                       name=f"gm20_{gi}")
                    nc.vector.tensor_mul(gm2[:], g[:, 0:4], g[:, 0:4])
                    vv = st_pool.tile([1, 4], F32, tag="vv", name=f"vv0_{gi}")
                    nc.vector.tensor_sub(vv[:], g[:, 4:8], gm2[:])
                    nc.vector.tensor_scalar_add(vv[:], vv[:], EPS)
                    yy = st_pool.tile([1, 4], F32, tag="yy", name=f"yy0_{gi}")
                    nc.vector.tensor_scalar(yy[:].bitcast(I32),
                                            vv[:].bitcast(I32),
                                            1, -1, op0=ALU.arith_shift_right,
                                            op1=ALU.bitwise_xor)
                    nc.vector.tensor_scalar_add(yy[:].bitcast(I32),
                                                yy[:].bitcast(I32), 0x5f3759e0)
                    invmean = st_pool.tile([1, 8], F32, tag="invmean",
                                           name=f"im0_{gi}")
                    tnr = st_pool.tile([1, 4], F32, tag="tnr",
                                       name=f"tnr0_{gi}")
                    for it in range(2):
                        nc.vector.tensor_mul(tnr[:], yy[:], yy[:])
                        nc.vector.tensor_mul(tnr[:], tnr[:], vv[:])
                        nc.vector.tensor_scalar(tnr[:], tnr[:], -0.5, 1.5,
                                                op0=ALU.mult, op1=ALU.add)
                        dst = yy[:] if it < 1 else invmean[:, 0:4]
                        nc.vector.tensor_mul(dst, yy[:], tnr[:])
                    nc.vector.tensor_copy(invmean[:, 4:8], g[:, 0:4])
                    ps_b = ps.tile([128, 8], F32, tag="smalls", bufs=2,
                                   name=f"psb0_{gi}")
                    nc.tensor.matmul(ps_b[:], ones_row[:], invmean[:],
                                     start=True, stop=True)
                    bbq = st_pool.tile([128, 8], F32, tag="bbq", bufs=3,
                                       name=f"bbq0_{gi}")
                    nc.vector.tensor_copy(bbq[:], ps_b[:])
                    for v in range(4):
                        u = gi * 4 + v
                        r, s = u // 2, u % 2
                        nc.vector.tensor_scalar(
                            fcq[:, r * T2 + s * T: r * T2 + (s + 1) * T],
                            fq[:, r * T2 + s * T: r * T2 + (s + 1) * T],
                            bbq[:, 4 + v:5 + v], bbq[:, v:v + 1],
                            op0=ALU.subtract, op1=ALU.mult)
                return fcq

            def fc_block(q, br, fcq, wq):
                b1pt, w2s, b2c, _ = cb[br]
                h1s = []
                for r in range(4):
                    slot = q * 4 + r
                    h1 = h1_pool.tile([128, NHC * T2], FP16, tag="h1")
                    h1s.append(h1)
                    for hc in range(NHC):
                        p2 = ps.tile([128, T2], F32, tag="ps2", bufs=3,
                                     name=f"p2_{br}_{slot}_{hc}")
                        for s in range(2):
                            for j in range(4):
                                nc.tensor.matmul(
                                    p2[32 * j:32 * j + 32, s * T:(s + 1) * T],
                                    wq[:, (r * NHC + hc) * 128 + 32 * j:
                                          (r * NHC + hc) * 128 + 32 * j + 32],
                                    fcq[:, r * T2 + s * T: r * T2 + (s + 1) * T],
                                    start=True, stop=True,
                                    tile_position=(0, 32 * j))
                        nc.scalar.activation(
                            h1[:, hc * T2:(hc + 1) * T2], p2[:], AF.Tanh,
                            bias=b1pt[:, slot * NHC + hc: slot * NHC + hc + 1])
                fq2 = ps.tile([128, T2], F32, tag="ps2", bufs=3,
                              name=f"fq2_{br}_{q}")
                for hc in range(NHC):
                    for r in range(4):
                        slot = q * 4 + r
                        for s in range(2):
                            nc.tensor.matmul(
                                fq2[32 * r:32 * r + 32, s * T:(s + 1) * T],
                                w2s[:, hc * NB * 32 + slot * 32:
                                       hc * NB * 32 + slot * 32 + 32],
                                h1s[r][:, hc * T2 + s * T: hc * T2 + (s + 1) * T],
                                start=(hc == 0), stop=(hc == NHC - 1),
                                tile_position=(0, 32 * r))
                gt = const_pool.tile([128, T2], FP16, tag=f"grp_{br}_{q}",
                                     name=f"grp_{br}_{q}")
                nc.scalar.activation(gt[:], fq2[:], AF.Tanh,
                                     bias=b2c[:, q:q + 1],
                                     scale=0.5 if br == "m" else 1.0)
                grp[(q, br)] = gt

            def assembly(q):
                _, _, _, noisy_m = cb["m"]
                _, _, _, noisy_p = cb["p"]
                nmag = noisy_m[:, q * T2:(q + 1) * T2]
                nph = noisy_p[:, q * T2:(q + 1) * T2]
                mask01 = asm_pool.tile([128, T2], FP16, tag="mask01")
                nc.vector.tensor_scalar(mask01[:], grp[(q, "m")][:], 0.5, 0.5,
                                        op0=ALU.mult, op1=ALU.add)
                enh = asm_pool.tile([128, T2], FP16, tag="enh")
                nc.vector.tensor_mul(enh[:], mask01[:], nmag)
                ang = asm_pool.tile([128, T2], F32, tag="ang")
                nc.vector.scalar_tensor_tensor(ang[:], grp[(q, "p")][:], PI,
                                               nph, op0=ALU.mult, op1=ALU.add)
                t2 = asm_pool.tile([128, T2], F32, tag="t2")
                nc.vector.tensor_scalar(t2[:], ang[:], INV2PI, MAGIC,
                                        op0=ALU.mult, op1=ALU.add)
                m2 = asm_pool.tile([128, T2], F32, tag="m2")
                nc.vector.tensor_scalar(m2[:], t2[:], MAGIC, N2PI,
                                        op0=ALU.subtract, op1=ALU.mult)
                ws = asm_pool.tile([128, T2], F32, tag="ws")
                nc.vector.tensor_add(ws[:], m2[:], ang[:])
                sn = asm_pool.tile([128, T2], FP16, tag="sn")
                nc.scalar.activation(sn[:], ws[:], AF.Sin)
                # cos(ws) = sin(pi/2 - |ws|); |ws| <= pi keeps Sin in-domain
                aws = asm_pool.tile([128, T2], F32, tag="aws")
                nc.vector.tensor_scalar(aws[:].bitcast(I32), ws[:].bitcast(I32),
                                        0x7FFFFFFF, 0,
                                        op0=ALU.bitwise_and, op1=ALU.bitwise_or)
                cn = asm_pool.tile([128, T2], FP16, tag="cn")
                nc.scalar.activation(cn[:], aws[:], AF.Sin, bias=halfpi[:],
                                     scale=-1.0)
                ot = asm_pool.tile([128, 2 * T2], F32, tag="ot")
                ot2 = ot[:].rearrange("p (st two) -> p st two", two=2)
                nc.vector.tensor_mul(ot2[:, :, 0], enh[:], cn[:])
                nc.vector.tensor_mul(ot2[:, :, 1], enh[:], sn[:])
                nc.sync.dma_start(out_d[:, q * 2 * T2:(q + 1) * 2 * T2], ot[:])

            # ---- software-pipelined main loop ----
            blocks = [(q, br) for q in range(NQ) for br in ("m", "p")]
            tiles = {0: load_block(*blocks[0])}
            fcqs = {}
            fcqs[0] = stats_fcent0(tiles[0][0])
            for i, (q, br) in enumerate(blocks):
                if i + 1 < len(blocks):
                    tiles[i + 1] = load_block(*blocks[i + 1])
                fc_block(q, br, fcqs[i], tiles[i][1])
                if i + 1 < len(blocks):
                    fcqs[i + 1] = stats_fcent(blocks[i + 1][0],
                                              blocks[i + 1][1],
                                              tiles[i + 1][0])
                # assembly of quad q-1 deferred: its DVE chain and Sin ops
                # hide under this block's tanh stream
                if br == "m" and q >= 1:
                    assembly(q - 1)
            assembly(NQ - 1)

    nc.compile()
    return nc


def _prep_core(bands, gamma, beta, W1, b1, W2, b2, halve_b2):
    """Host-side constant prep for one (core, branch). bands: list of global
    band indices (len<=16; missing slots are zero/dummy)."""
    nb = len(bands)
    W1g = W1 * gamma[:, None, :]                     # [K, H, C]
    w1 = np.zeros((C, NB * H), np.float16)
    for sl, k in enumerate(bands):
        w1[:, sl * H:(sl + 1) * H] = W1g[k].T.astype(np.float16)
    b1p = b1 + np.einsum('khc,kc->kh', W1, beta)     # [K, H]
    b1pt = np.zeros((128, NB * NHC), np.float32)
    for sl, k in enumerate(bands):
        for hc in range(NHC):
            b1pt[:, sl * NHC + hc] = b1p[k, hc * 128:(hc + 1) * 128]
    w2s = np.zeros((128, NHC * NB * 32), np.float16)
    for sl, k in enumerate(bands):
        w, off = BANDS[k], int(OFFS[k])
        for hc in range(NHC):
            w2s[:, hc * NB * 32 + sl * 32: hc * NB * 32 + sl * 32 + w] = \
                W2[off:off + w, hc * 128:(hc + 1) * 128].T.astype(np.float16)
    b2c = np.zeros((128, NQ), np.float32)
    for sl, k in enumerate(bands):
        qq, r = sl // 4, sl % 4
        w, off = BANDS[k], int(OFFS[k])
        b2c[32 * r:32 * r + w, qq] = b2[off:off + w]
    if halve_b2:
        b2c *= 0.5
    return w1, b1pt, w2s, b2c


def _noisy_strips(bands, arr2):
    """arr2: [2, F, T] noisy for the pair's two samples -> strip layout
    [128, NQ*T2] fp16."""
    outp = np.zeros((128, NQ * T2), np.float16)
    for sl, k in enumerate(bands):
        qq, r = sl // 4, sl % 4
        w, off = BANDS[k], int(OFFS[k])
        for s in range(2):
            outp[32 * r:32 * r + w, qq * T2 + s * T:qq * T2 + (s + 1) * T] = \
                arr2[s, off:off + w, :].astype(np.float16)
    return outp


def kernel(mag_features, phase_features, noisy_mag, noisy_phase,
           mag_gamma, mag_beta, mag_W1, mag_b1, mag_W2, mag_b2,
           ph_gamma, ph_beta, ph_W1, ph_b1, ph_W2, ph_b2):
    if "nc" not in _cache:
        _cache["nc"] = _build()
    nc = _cache["nc"]

    mag_features = np.asarray(mag_features)
    phase_features = np.asarray(phase_features)
    noisy_mag = np.asarray(noisy_mag)
    noisy_phase = np.asarray(noisy_phase)
    prm = {
        "m": tuple(np.asarray(x) for x in
                   (mag_gamma, mag_beta, mag_W1, mag_b1, mag_W2, mag_b2)),
        "p": tuple(np.asarray(x) for x in
                   (ph_gamma, ph_beta, ph_W1, ph_b1, ph_W2, ph_b2)),
    }
    feats = {"m": mag_features, "p": phase_features}
    noisy = {"m": noisy_mag, "p": noisy_phase}

    band_sets = [list(range(0, 16)), list(range(16, 31))]
    shared = dict(
        ones_col=np.ones((128, 1), np.float32),
        ones_row=np.ones((1, 128), np.float32),
        halfpi=np.full((128, 1), np.pi / 2, np.float32),
    )
    # per-(half, branch) weight prep (same for every pair)
    wprep = {}
    for half in range(2):
        for br in ("m", "p"):
            g, be, W1_, b1_, W2_, b2_ = prm[br]
            wprep[(half, br)] = _prep_core(band_sets[half], g, be, W1_, b1_,
                                           W2_, b2_, halve_b2=(br == "m"))

    in_maps = []
    for core in range(8):
        pair, half = core // 2, core % 2
        sA, sB = 2 * pair, 2 * pair + 1
        bands = band_sets[half]
        m = dict(shared)
        for br in ("m", "p"):
            w1, b1pt, w2s, b2c = wprep[(half, br)]
            m[f"w1_{br}"], m[f"b1pt_{br}"] = w1, b1pt
            m[f"w2s_{br}"], m[f"b2c_{br}"] = w2s, b2c
            # features: [2, C, T, nb] -> [C, nb, 2, T] -> [128, NB*T2]
            fa = feats[br][[sA, sB]][:, :, :, bands]        # [2,C,T,nb]
            fa = np.ascontiguousarray(fa.transpose(1, 3, 0, 2))  # [C,nb,2,T]
            ft = np.zeros((C, NB * T2), np.float16)
            ft[:, :fa.shape[1] * T2] = fa.reshape(C, -1).astype(np.float16)
            m[f"feat_{br}"] = ft
            m[f"noisy_{br}"] = _noisy_strips(bands, noisy[br][[sA, sB]])
        in_maps.append(m)

    import os
    trace = bool(os.environ.get("BASS_PROFILE"))
    res = run_bass_kernel_spmd(nc, in_maps, list(range(8)), trace=trace)
    _cache["last_result"] = res

    out = np.zeros((B, F, T), np.complex64)
    for core in range(8):
        pair, half = core // 2, core % 2
        sA, sB = 2 * pair, 2 * pair + 1
        bands = band_sets[half]
        oc = res.results[core]["out"].reshape(128, NQ, 2, T, 2)
        occ = (oc[..., 0] + 1j * oc[..., 1]).astype(np.complex64)
        for sl, k in enumerate(bands):
            qq, r = sl // 4, sl % 4
            w, off = BANDS[k], int(OFFS[k])
            out[sA, off:off + w] = occ[32 * r:32 * r + w, qq, 0]
            out[sB, off:off + w] = occ[32 * r:32 * r + w, qq, 1]
    return out


# revision 27
# speedup vs baseline: 1.0063x; 1.0035x over previous
"""Trainium2 Bass kernel for nn_DualBandDecoder (v2).

Sharding: core pair p = (2p, 2p+1) shares samples (2p, 2p+1); even core
handles bands 0..15, odd core bands 16..30 (+1 dummy slot). Each core
processes 16 band-slots x 2 samples x 2 branches.

Per (slot, hc) the fc1 output for BOTH samples lands in one 2-bank PSUM
tile [128, 1024], so one ACTIVATE tanh covers 1024 elements with a single
per-partition bias (b1 is sample-independent after normalizing features on
DVE). fc1 matmuls are 4-way column-strip tiled (concurrent in the PE
array); fc2 is hc-outer so the 4 bands of a quad overlap. All activation
functions are Tanh/Sin (sigmoid via tanh identity), which live in ONE act
table set -> single table load, assembly pipelines with compute.
"""
import sys
sys.path.insert(0, '/opt/trn_rl_repo')

import numpy as np

import concourse.bacc as bacc
import concourse.tile as tile
import concourse.mybir as mybir
from concourse.bass_utils import run_bass_kernel_spmd

F32 = mybir.dt.float32
FP16 = mybir.dt.float16
I32 = mybir.dt.int32
AF = mybir.ActivationFunctionType
ALU = mybir.AluOpType

# problem constants
B, C, T = 8, 128, 512
BANDS = [2] + [3] * 10 + [8] * 12 + [16] * 7 + [17]
K = len(BANDS)                      # 31
F = sum(BANDS)                      # 257
H = 4 * C                           # 512
NHC = 4
EPS = 1e-5
OFFS = np.concatenate([[0], np.cumsum(BANDS)]).astype(int)

NB = 16                             # band-slots per core
NQ = 4                              # quads per core
T2 = 2 * T                          # 1024: two samples' t-cols

MAGIC = float(1.5 * 2 ** 23)
INV2PI = float(1.0 / (2 * np.pi))
N2PI = float(-2 * np.pi)
PI = float(np.pi)

_cache = {}


def _build():
    nc = bacc.Bacc("TRN2", target_bir_lowering=False)

    ins = {}
    for br in ("m", "p"):
        ins[f"feat_{br}"] = nc.dram_tensor(f"feat_{br}", [C, NB * T2], FP16,
                                           kind="ExternalInput")
        ins[f"w1_{br}"] = nc.dram_tensor(f"w1_{br}", [C, NB * H], FP16,
                                         kind="ExternalInput")
        ins[f"b1pt_{br}"] = nc.dram_tensor(f"b1pt_{br}", [128, NB * NHC], F32,
                                           kind="ExternalInput")
        ins[f"w2s_{br}"] = nc.dram_tensor(f"w2s_{br}", [128, NHC * NB * 32],
                                          FP16, kind="ExternalInput")
        ins[f"b2c_{br}"] = nc.dram_tensor(f"b2c_{br}", [128, NQ], F32,
                                          kind="ExternalInput")
        ins[f"noisy_{br}"] = nc.dram_tensor(f"noisy_{br}", [128, NQ * T2],
                                            FP16, kind="ExternalInput")
    ones_col_d = nc.dram_tensor("ones_col", [128, 1], F32, kind="ExternalInput")
    ones_row_d = nc.dram_tensor("ones_row", [1, 128], F32, kind="ExternalInput")
    halfpi_d = nc.dram_tensor("halfpi", [128, 1], F32, kind="ExternalInput")
    out_d = nc.dram_tensor("out", [128, NQ * 2 * T2], F32,
                           kind="ExternalOutput")

    with tile.TileContext(nc) as tc:
        with (
            tc.tile_pool(name="featq", bufs=3) as feat_pool,
            tc.tile_pool(name="fcq", bufs=2) as fc_pool,
            tc.tile_pool(name="w1q", bufs=2) as w1_pool,
            tc.tile_pool(name="h1sb", bufs=5) as h1_pool,
            tc.tile_pool(name="const", bufs=1) as const_pool,
            tc.tile_pool(name="stats", bufs=2) as st_pool,
            tc.tile_pool(name="asm", bufs=1) as asm_pool,
            tc.tile_pool(name="mainps", bufs=1, space="PSUM") as ps,
        ):
            # ---- single act-table load: silu_and_others has Tanh AND Sin ----
            tl = mybir.InstLoadActFuncSet(
                name=nc.get_next_instruction_name(), act_func_set_id=18,
                ins=[], outs=[])
            nc.scalar.add_instruction(tl)

            # ---- critical path first: quad-0 mag tiles, split DMAs on
            # two queues (sync: features, gpsimd: weights) ----
            fq0 = feat_pool.tile([128, 4 * T2], FP16, tag="fq", name="fq_m_0")
            nc.sync.dma_start(fq0[:, 0:2 * T2], ins["feat_m"][:, 0:2 * T2])
            wq0 = w1_pool.tile([128, 4 * H], FP16, tag="w1q", name="w1q_m_0")
            nc.gpsimd.dma_start(wq0[:, 0:2 * H], ins["w1_m"][:, 0:2 * H])
            ones_col = const_pool.tile([128, 1], F32)
            nc.sync.dma_start(ones_col[:], ones_col_d[:])
            ones_row = const_pool.tile([1, 128], F32)
            nc.sync.dma_start(ones_row[:], ones_row_d[:])
            b1pt_m = const_pool.tile([128, NB * NHC], F32, tag="b1_m")
            nc.gpsimd.dma_start(b1pt_m[:], ins["b1pt_m"][:])
            nc.sync.dma_start(fq0[:, 2 * T2:4 * T2],
                              ins["feat_m"][:, 2 * T2:4 * T2])
            nc.gpsimd.dma_start(wq0[:, 2 * H:4 * H], ins["w1_m"][:, 2 * H:4 * H])

            # ---- remaining constants ----
            halfpi = const_pool.tile([128, 1], F32)
            nc.sync.dma_start(halfpi[:], halfpi_d[:])
            cb = {}
            for br in ("m", "p"):
                if br == "m":
                    b1pt = b1pt_m
                else:
                    b1pt = const_pool.tile([128, NB * NHC], F32, tag="b1_p")
                    nc.sync.dma_start(b1pt[:], ins["b1pt_p"][:])
                w2s = const_pool.tile([128, NHC * NB * 32], FP16, tag=f"w2_{br}")
                nc.sync.dma_start(w2s[:], ins[f"w2s_{br}"][:])
                b2c = const_pool.tile([128, NQ], F32, tag=f"b2_{br}")
                nc.sync.dma_start(b2c[:], ins[f"b2c_{br}"][:])
                noisy = const_pool.tile([128, NQ * T2], FP16, tag=f"no_{br}")
                nc.gpsimd.dma_start(noisy[:], ins[f"noisy_{br}"][:])
                cb[br] = (b1pt, w2s, b2c, noisy)

            grp = {}

            def load_block(q, br):
                if q == 0 and br == "m":
                    return fq0, wq0
                fq = feat_pool.tile([128, 4 * T2], FP16, tag="fq",
                                    name=f"fq_{br}_{q}")
                nc.sync.dma_start(
                    fq[:], ins[f"feat_{br}"][:, q * 4 * T2:(q + 1) * 4 * T2])
                wq = w1_pool.tile([128, 4 * H], FP16, tag="w1q",
                                  name=f"w1q_{br}_{q}")
                nc.sync.dma_start(
                    wq[:], ins[f"w1_{br}"][:, q * 4 * H:(q + 1) * 4 * H])
                return fq, wq

            def stats_fcent(q, br, fq):
                st = st_pool.tile([128, 8 * 6], F32, tag="st",
                                  name=f"st_{br}_{q}")
                ag = st_pool.tile([128, 8 * 2], F32, tag="ag",
                                  name=f"ag_{br}_{q}")
                for u in range(8):
                    r, s = u // 2, u % 2
                    nc.vector.bn_stats(st[:, u * 6:(u + 1) * 6],
                                       fq[:, r * T2 + s * T: r * T2 + (s + 1) * T])
                    nc.vector.bn_aggr(ag[:, u * 2:(u + 1) * 2],
                                      st[:, u * 6:(u + 1) * 6])
                ag3 = ag[:].rearrange("c (u two) -> c u two", two=2)
                mean_ap = ag3[:, :, 0]
                var_ap = ag3[:, :, 1]
                sums = st_pool.tile([128, 16], F32, tag="sums",
                                    name=f"sums_{br}_{q}")
                nc.vector.tensor_copy(sums[:, 0:8], mean_ap)
                tmp = st_pool.tile([128, 8], F32, tag="tmp",
                                   name=f"tmp_{br}_{q}")
                nc.vector.tensor_mul(tmp[:], mean_ap, mean_ap)
                nc.vector.tensor_add(sums[:, 8:16], tmp[:], var_ap)
                ps_s = ps.tile([1, 16], F32, tag="smalls", bufs=2,
                               name=f"pss_{br}_{q}")
                nc.tensor.matmul(ps_s[:], ones_col[:], sums[:],
                                 start=True, stop=True)
                g = st_pool.tile([1, 16], F32, tag="g", name=f"g_{br}_{q}")
                nc.vector.tensor_scalar_mul(g[:], ps_s[:], 1.0 / C)
                gm2 = st_pool.tile([1, 8], F32, tag="gm2", name=f"gm2_{br}_{q}")
                nc.vector.tensor_mul(gm2[:], g[:, 0:8], g[:, 0:8])
                vv = st_pool.tile([1, 8], F32, tag="vv", name=f"vv_{br}_{q}")
                nc.vector.tensor_sub(vv[:], g[:, 8:16], gm2[:])
                nc.vector.tensor_scalar_add(vv[:], vv[:], EPS)
                yy = st_pool.tile([1, 8], F32, tag="yy", name=f"yy_{br}_{q}")
                nc.vector.tensor_scalar(yy[:].bitcast(I32), vv[:].bitcast(I32),
                                        1, -1, op0=ALU.arith_shift_right,
                                        op1=ALU.bitwise_xor)
                nc.vector.tensor_scalar_add(yy[:].bitcast(I32),
                                            yy[:].bitcast(I32), 0x5f3759e0)
                invmean = st_pool.tile([1, 16], F32, tag="invmean",
                                       name=f"im_{br}_{q}")
                tnr = st_pool.tile([1, 8], F32, tag="tnr", name=f"tnr_{br}_{q}")
                for it in range(2):
                    nc.vector.tensor_mul(tnr[:], yy[:], yy[:])
                    nc.vector.tensor_mul(tnr[:], tnr[:], vv[:])
                    nc.vector.tensor_scalar(tnr[:], tnr[:], -0.5, 1.5,
                                            op0=ALU.mult, op1=ALU.add)
                    dst = yy[:] if it < 1 else invmean[:, 0:8]
                    nc.vector.tensor_mul(dst, yy[:], tnr[:])
                nc.vector.tensor_copy(invmean[:, 8:16], g[:, 0:8])
                ps_b = ps.tile([128, 16], F32, tag="smalls", bufs=2,
                               name=f"psb_{br}_{q}")
                nc.tensor.matmul(ps_b[:], ones_row[:], invmean[:],
                                 start=True, stop=True)
                bbq = st_pool.tile([128, 16], F32, tag="bbq", bufs=3,
                                   name=f"bbq_{br}_{q}")
                nc.vector.tensor_copy(bbq[:], ps_b[:])
                fcq = fc_pool.tile([128, 4 * T2], FP16, tag="fcq",
                                   name=f"fcq_{br}_{q}")
                for u in range(8):
                    r, s = u // 2, u % 2
                    nc.vector.tensor_scalar(
                        fcq[:, r * T2 + s * T: r * T2 + (s + 1) * T],
                        fq[:, r * T2 + s * T: r * T2 + (s + 1) * T],
                        bbq[:, 8 + u:9 + u], bbq[:, u:u + 1],
                        op0=ALU.subtract, op1=ALU.mult)
                return fcq

            def stats_fcent0(fq):
                """Block (0,m): stats in two 2-slot groups so fc1 of slot 0
                starts before slots 2-3 features even land."""
                fcq = fc_pool.tile([128, 4 * T2], FP16, tag="fcq",
                                   name="fcq_m_0")
                for gi in range(2):
                    st = st_pool.tile([128, 4 * 6], F32, tag="st",
                                      name=f"st0_{gi}")
                    ag = st_pool.tile([128, 4 * 2], F32, tag="ag",
                                      name=f"ag0_{gi}")
                    for v in range(4):
                        u = gi * 4 + v
                        r, s = u // 2, u % 2
                        nc.vector.bn_stats(st[:, v * 6:(v + 1) * 6],
                                           fq[:, r * T2 + s * T: r * T2 + (s + 1) * T])
                        nc.vector.bn_aggr(ag[:, v * 2:(v + 1) * 2],
                                          st[:, v * 6:(v + 1) * 6])
                    ag3 = ag[:].rearrange("c (u two) -> c u two", two=2)
                    mean_ap = ag3[:, :, 0]
                    var_ap = ag3[:, :, 1]
                    sums = st_pool.tile([128, 8], F32, tag="sums",
                                        name=f"sums0_{gi}")
                    nc.vector.tensor_copy(sums[:, 0:4], mean_ap)
                    tmp = st_pool.tile([128, 4], F32, tag="tmp",
                                       name=f"tmp0_{gi}")
                    nc.vector.tensor_mul(tmp[:], mean_ap, mean_ap)
                    nc.vector.tensor_add(sums[:, 4:8], tmp[:], var_ap)
                    ps_s = ps.tile([1, 8], F32, tag="smalls", bufs=2,
                                   name=f"pss0_{gi}")
                    nc.tensor.matmul(ps_s[:], ones_col[:], sums[:],
                                     start=True, stop=True)
                    g = st_pool.tile([1, 8], F32, tag="g", name=f"g0_{gi}")
                    nc.vector.tensor_scalar_mul(g[:], ps_s[:], 1.0 / C)
                    gm2 = st_pool.tile([1, 4], F32, tag="gm2",
                                       name=f"gm20_{gi}")
                    nc.vector.tensor_mul(gm2[:], g[:, 0:4], g[:, 0:4])
                    vv = st_pool.tile([1, 4], F32, tag="vv", name=f"vv0_{gi}")
                    nc.vector.tensor_sub(vv[:], g[:, 4:8], gm2[:])
                    nc.vector.tensor_scalar_add(vv[:], vv[:], EPS)
                    yy = st_pool.tile([1, 4], F32, tag="yy", name=f"yy0_{gi}")
                    nc.vector.tensor_scalar(yy[:].bitcast(I32),
                                            vv[:].bitcast(I32),
                                            1, -1, op0=ALU.arith_shift_right,
                                            op1=ALU.bitwise_xor)
                    nc.vector.tensor_scalar_add(yy[:].bitcast(I32),
                                                yy[:].bitcast(I32), 0x5f3759e0)
                    invmean = st_pool.tile([1, 8], F32, tag="invmean",
                                           name=f"im0_{gi}")
                    tnr = st_pool.tile([1, 4], F32, tag="tnr",
                                       name=f"tnr0_{gi}")
                    for it in range(2):
                        nc.vector.tensor_mul(tnr[:], yy[:], yy[:])
                        nc.vector.tensor_mul(tnr[:], tnr[:], vv[:])
                        nc.vector.tensor_scalar(tnr[:], tnr[:], -0.5, 1.5,
                                                op0=ALU.mult, op1=ALU.add)
                        dst = yy[:] if it < 1 else invmean[:, 0:4]
                        nc.vector.tensor_mul(dst, yy[:], tnr[:])
                    nc.vector.tensor_copy(invmean[:, 4:8], g[:, 0:4])
                    ps_b = ps.tile([128, 8], F32, tag="smalls", bufs=2,
                                   name=f"psb0_{gi}")
                    nc.tensor.matmul(ps_b[:], ones_row[:], invmean[:],
                                     start=True, stop=True)
                    bbq = st_pool.tile([128, 8], F32, tag="bbq", bufs=3,
                                       name=f"bbq0_{gi}")
                    nc.vector.tensor_copy(bbq[:], ps_b[:])
                    for v in range(4):
                        u = gi * 4 + v
                        r, s = u // 2, u % 2
                        nc.vector.tensor_scalar(
                            fcq[:, r * T2 + s * T: r * T2 + (s + 1) * T],
                            fq[:, r * T2 + s * T: r * T2 + (s + 1) * T],
                            bbq[:, 4 + v:5 + v], bbq[:, v:v + 1],
                            op0=ALU.subtract, op1=ALU.mult)
                return fcq

            pending_grp = []

            def flush_grp():
                while pending_grp:
                    fq2p, brp, qp, b2cp = pending_grp.pop(0)
                    gt = const_pool.tile([128, T2], FP16, tag=f"grp_{brp}_{qp}",
                                         name=f"grp_{brp}_{qp}")
                    nc.scalar.activation(gt[:], fq2p[:], AF.Tanh,
                                         bias=b2cp[:, qp:qp + 1],
                                         scale=0.5 if brp == "m" else 1.0)
                    grp[(qp, brp)] = gt

            def fc_block(q, br, fcq, wq):
                b1pt, w2s, b2c, _ = cb[br]
                h1s = []
                for r in range(4):
                    slot = q * 4 + r
                    h1 = h1_pool.tile([128, NHC * T2], FP16, tag="h1")
                    h1s.append(h1)
                    if r == 1:
                        flush_grp()
                    for hc in range(NHC):
                        p2 = ps.tile([128, T2], F32, tag="ps2", bufs=3,
                                     name=f"p2_{br}_{slot}_{hc}")
                        for s in range(2):
                            for j in range(4):
                                nc.tensor.matmul(
                                    p2[32 * j:32 * j + 32, s * T:(s + 1) * T],
                                    wq[:, (r * NHC + hc) * 128 + 32 * j:
                                          (r * NHC + hc) * 128 + 32 * j + 32],
                                    fcq[:, r * T2 + s * T: r * T2 + (s + 1) * T],
                                    start=True, stop=True,
                                    tile_position=(0, 32 * j))
                        nc.scalar.activation(
                            h1[:, hc * T2:(hc + 1) * T2], p2[:], AF.Tanh,
                            bias=b1pt[:, slot * NHC + hc: slot * NHC + hc + 1])
                fq2 = ps.tile([128, T2], F32, tag="ps2", bufs=3,
                              name=f"fq2_{br}_{q}")
                for hc in range(NHC):
                    for r in range(4):
                        slot = q * 4 + r
                        for s in range(2):
                            nc.tensor.matmul(
                                fq2[32 * r:32 * r + 32, s * T:(s + 1) * T],
                                w2s[:, hc * NB * 32 + slot * 32:
                                       hc * NB * 32 + slot * 32 + 32],
                                h1s[r][:, hc * T2 + s * T: hc * T2 + (s + 1) * T],
                                start=(hc == 0), stop=(hc == NHC - 1),
                                tile_position=(0, 32 * r))
                pending_grp.append((fq2, br, q, b2c))

            def assembly(q):
                _, _, _, noisy_m = cb["m"]
                _, _, _, noisy_p = cb["p"]
                nmag = noisy_m[:, q * T2:(q + 1) * T2]
                nph = noisy_p[:, q * T2:(q + 1) * T2]
                mask01 = asm_pool.tile([128, T2], FP16, tag="mask01")
                nc.vector.tensor_scalar(mask01[:], grp[(q, "m")][:], 0.5, 0.5,
                                        op0=ALU.mult, op1=ALU.add)
                enh = asm_pool.tile([128, T2], FP16, tag="enh")
                nc.vector.tensor_mul(enh[:], mask01[:], nmag)
                ang = asm_pool.tile([128, T2], F32, tag="ang")
                nc.vector.scalar_tensor_tensor(ang[:], grp[(q, "p")][:], PI,
                                               nph, op0=ALU.mult, op1=ALU.add)
                t2 = asm_pool.tile([128, T2], F32, tag="t2")
                nc.vector.tensor_scalar(t2[:], ang[:], INV2PI, MAGIC,
                                        op0=ALU.mult, op1=ALU.add)
                m2 = asm_pool.tile([128, T2], F32, tag="m2")
                nc.vector.tensor_scalar(m2[:], t2[:], MAGIC, N2PI,
                                        op0=ALU.subtract, op1=ALU.mult)
                ws = asm_pool.tile([128, T2], F32, tag="ws")
                nc.vector.tensor_add(ws[:], m2[:], ang[:])
                sn = asm_pool.tile([128, T2], FP16, tag="sn")
                nc.scalar.activation(sn[:], ws[:], AF.Sin)
                # cos(ws) = sin(pi/2 - |ws|); |ws| <= pi keeps Sin in-domain
                aws = asm_pool.tile([128, T2], F32, tag="aws")
                nc.vector.tensor_scalar(aws[:].bitcast(I32), ws[:].bitcast(I32),
                                        0x7FFFFFFF, 0,
                                        op0=ALU.bitwise_and, op1=ALU.bitwise_or)
                cn = asm_pool.tile([128, T2], FP16, tag="cn")
                nc.scalar.activation(cn[:], aws[:], AF.Sin, bias=halfpi[:],
                                     scale=-1.0)
                ot = asm_pool.tile([128, 2 * T2], F32, tag="ot")
                ot2 = ot[:].rearrange("p (st two) -> p st two", two=2)
                nc.vector.tensor_mul(ot2[:, :, 0], enh[:], cn[:])
                nc.vector.tensor_mul(ot2[:, :, 1], enh[:], sn[:])
                nc.sync.dma_start(out_d[:, q * 2 * T2:(q + 1) * 2 * T2], ot[:])

            # ---- software-pipelined main loop ----
            blocks = [(q, br) for q in range(NQ) for br in ("m", "p")]
            tiles = {0: load_block(*blocks[0])}
            fcqs = {}
            fcqs[0] = stats_fcent0(tiles[0][0])
            for i, (q, br) in enumerate(blocks):
                if i + 1 < len(blocks):
                    tiles[i + 1] = load_block(*blocks[i + 1])
                fc_block(q, br, fcqs[i], tiles[i][1])
                if i + 1 < len(blocks):
                    fcqs[i + 1] = stats_fcent(blocks[i + 1][0],
                                              blocks[i + 1][1],
                                              tiles[i + 1][0])
                # assembly of quad q-1 deferred: its DVE chain and Sin ops
                # hide under this block's tanh stream
                if br == "m" and q >= 1:
                    assembly(q - 1)
            flush_grp()
            assembly(NQ - 1)

    nc.compile()
    return nc


def _prep_core(bands, gamma, beta, W1, b1, W2, b2, halve_b2):
    """Host-side constant prep for one (core, branch). bands: list of global
    band indices (len<=16; missing slots are zero/dummy)."""
    nb = len(bands)
    W1g = W1 * gamma[:, None, :]                     # [K, H, C]
    w1 = np.zeros((C, NB * H), np.float16)
    for sl, k in enumerate(bands):
        w1[:, sl * H:(sl + 1) * H] = W1g[k].T.astype(np.float16)
    b1p = b1 + np.einsum('khc,kc->kh', W1, beta)     # [K, H]
    b1pt = np.zeros((128, NB * NHC), np.float32)
    for sl, k in enumerate(bands):
        for hc in range(NHC):
            b1pt[:, sl * NHC + hc] = b1p[k, hc * 128:(hc + 1) * 128]
    w2s = np.zeros((128, NHC * NB * 32), np.float16)
    for sl, k in enumerate(bands):
        w, off = BANDS[k], int(OFFS[k])
        for hc in range(NHC):
            w2s[:, hc * NB * 32 + sl * 32: hc * NB * 32 + sl * 32 + w] = \
                W2[off:off + w, hc * 128:(hc + 1) * 128].T.astype(np.float16)
    b2c = np.zeros((128, NQ), np.float32)
    for sl, k in enumerate(bands):
        qq, r = sl // 4, sl % 4
        w, off = BANDS[k], int(OFFS[k])
        b2c[32 * r:32 * r + w, qq] = b2[off:off + w]
    if halve_b2:
        b2c *= 0.5
    return w1, b1pt, w2s, b2c


def _noisy_strips(bands, arr2):
    """arr2: [2, F, T] noisy for the pair's two samples -> strip layout
    [128, NQ*T2] fp16."""
    outp = np.zeros((128, NQ * T2), np.float16)
    for sl, k in enumerate(bands):
        qq, r = sl // 4, sl % 4
        w, off = BANDS[k], int(OFFS[k])
        for s in range(2):
            outp[32 * r:32 * r + w, qq * T2 + s * T:qq * T2 + (s + 1) * T] = \
                arr2[s, off:off + w, :].astype(np.float16)
    return outp


def kernel(mag_features, phase_features, noisy_mag, noisy_phase,
           mag_gamma, mag_beta, mag_W1, mag_b1, mag_W2, mag_b2,
           ph_gamma, ph_beta, ph_W1, ph_b1, ph_W2, ph_b2):
    if "nc" not in _cache:
        _cache["nc"] = _build()
    nc = _cache["nc"]

    mag_features = np.asarray(mag_features)
    phase_features = np.asarray(phase_features)
    noisy_mag = np.asarray(noisy_mag)
    noisy_phase = np.asarray(noisy_phase)
    prm = {
        "m": tuple(np.asarray(x) for x in
                   (mag_gamma, mag_beta, mag_W1, mag_b1, mag_W2, mag_b2)),
        "p": tuple(np.asarray(x) for x in
                   (ph_gamma, ph_beta, ph_W1, ph_b1, ph_W2, ph_b2)),
    }
    feats = {"m": mag_features, "p": phase_features}
    noisy = {"m": noisy_mag, "p": noisy_phase}

    band_sets = [list(range(0, 16)), list(range(16, 31))]
    shared = dict(
        ones_col=np.ones((128, 1), np.float32),
        ones_row=np.ones((1, 128), np.float32),
        halfpi=np.full((128, 1), np.pi / 2, np.float32),
    )
    # per-(half, branch) weight prep (same for every pair)
    wprep = {}
    for half in range(2):
        for br in ("m", "p"):
            g, be, W1_, b1_, W2_, b2_ = prm[br]
            wprep[(half, br)] = _prep_core(band_sets[half], g, be, W1_, b1_,
                                           W2_, b2_, halve_b2=(br == "m"))

    in_maps = []
    for core in range(8):
        pair, half = core // 2, core % 2
        sA, sB = 2 * pair, 2 * pair + 1
        bands = band_sets[half]
        m = dict(shared)
        for br in ("m", "p"):
            w1, b1pt, w2s, b2c = wprep[(half, br)]
            m[f"w1_{br}"], m[f"b1pt_{br}"] = w1, b1pt
            m[f"w2s_{br}"], m[f"b2c_{br}"] = w2s, b2c
            # features: [2, C, T, nb] -> [C, nb, 2, T] -> [128, NB*T2]
            fa = feats[br][[sA, sB]][:, :, :, bands]        # [2,C,T,nb]
            fa = np.ascontiguousarray(fa.transpose(1, 3, 0, 2))  # [C,nb,2,T]
            ft = np.zeros((C, NB * T2), np.float16)
            ft[:, :fa.shape[1] * T2] = fa.reshape(C, -1).astype(np.float16)
            m[f"feat_{br}"] = ft
            m[f"noisy_{br}"] = _noisy_strips(bands, noisy[br][[sA, sB]])
        in_maps.append(m)

    import os
    trace = bool(os.environ.get("BASS_PROFILE"))
    res = run_bass_kernel_spmd(nc, in_maps, list(range(8)), trace=trace)
    _cache["last_result"] = res

    out = np.zeros((B, F, T), np.complex64)
    for core in range(8):
        pair, half = core // 2, core % 2
        sA, sB = 2 * pair, 2 * pair + 1
        bands = band_sets[half]
        oc = res.results[core]["out"].reshape(128, NQ, 2, T, 2)
        occ = (oc[..., 0] + 1j * oc[..., 1]).astype(np.complex64)
        for sl, k in enumerate(bands):
            qq, r = sl // 4, sl % 4
            w, off = BANDS[k], int(OFFS[k])
            out[sA, off:off + w] = occ[32 * r:32 * r + w, qq, 0]
            out[sB, off:off + w] = occ[32 * r:32 * r + w, qq, 1]
    return out
